# revision 11
# baseline (speedup 1.0000x reference)
"""Trainium2 Bass kernel for per-series OLS trend extrapolation.

Math: out[b, c] = sum_w g[w] * x[b, w, c], where
  g[w] = 1/W + (w - t_mean) * (t_pred - t_mean) / sum((w - t_mean)^2)

i.e. a single fixed weighted reduction along the window axis. Pure data
parallel: batch (256) sharded 32-per-core across 8 cores.

Device kernel (per core): x is cast to fp16 host-side (halves HBM traffic;
norm rel err ~3e-4). The reduction runs entirely on the tensor engine:
K = 128 = 32 batches x 4 window steps, M = 32 (batches), accumulating over
16 window segments into PSUM, then PSUM -> DRAM.
"""

import numpy as np

B, W, C = 256, 64, 3142
NCORES = 8
BPC = B // NCORES  # 32 batches per core
WSEG = 4           # window steps per segment (32 b * 4 w = 128 = K)
NSEG = W // WSEG   # 16 segments
NCHUNK = (C + 511) // 512  # 7 column chunks (psum bank = 512 fp32)

_cache = {}


def _build_program():
    import concourse.bacc as bacc
    import concourse.mybir as mybir
    import concourse.tile as tile

    fp16 = mybir.dt.float16
    f32 = mybir.dt.float32

    nc = bacc.Bacc("TRN2", target_bir_lowering=False, debug=False,
                   enable_asserts=False, num_devices=NCORES)
    x_ap = nc.dram_tensor("x", [BPC, W, C], fp16, kind="ExternalInput").ap()
    coef_ap = nc.dram_tensor("coef", [128, NSEG * BPC], fp16,
                             kind="ExternalInput").ap()
    out_ap = nc.dram_tensor("out", [BPC, C], f32, kind="ExternalOutput").ap()

    # x viewed per segment: [s, b, w, c]; DMA flattens (b, w) onto the 128
    # partitions of the destination tile, so partition k = b*WSEG + w.
    x_seg = x_ap.rearrange("b (s w) c -> s b w c", s=NSEG)

    with tile.TileContext(nc) as tc:
        with (
            tc.tile_pool(name="xp", bufs=6) as xp,
            tc.tile_pool(name="cp", bufs=1) as cp,
            tc.tile_pool(name="pp", bufs=1, space="PSUM") as pp,
        ):
            coef_sb = cp.tile([128, NSEG * BPC], fp16)
            # SWDGE queue: keeps the sync sequencer free for the x stream
            nc.gpsimd.dma_start(coef_sb[:], coef_ap[:])

            # one PSUM tensor spanning 7 banks; chunk j is bank j
            psum = pp.tile([BPC, NCHUNK * 512], f32, name="psum")

            CHALF = 3 * 512  # first 3 chunks / last 4 chunks
            for s in range(NSEG):
                xt = xp.tile([128, C], fp16)
                if s == NSEG - 1:
                    # split the final segment so its matmuls finish sooner
                    nc.sync.dma_start(xt[:, :CHALF], x_seg[s][:, :, :CHALF])
                    nc.sync.dma_start(xt[:, CHALF:], x_seg[s][:, :, CHALF:])
                else:
                    nc.sync.dma_start(xt[:], x_seg[s])
                for j in range(NCHUNK):
                    n = min(512, C - j * 512)
                    nc.tensor.matmul(
                        psum[:, j * 512:j * 512 + n],
                        coef_sb[:, s * BPC:(s + 1) * BPC],
                        xt[:, j * 512:j * 512 + n],
                        start=(s == 0),
                        stop=(s == NSEG - 1),
                    )

            # drain: copy+store pipelined in four slices
            out_sb = cp.tile([BPC, C], f32, name="out_sb")
            bounds = [0, 1024, 2048, 2560, C]
            for a, b in zip(bounds[:-1], bounds[1:]):
                nc.vector.tensor_copy(out_sb[:, a:b], psum[:, a:b])
                nc.sync.dma_start(out_ap[:, a:b], out_sb[:, a:b])

    nc.compile()
    return nc


def _get_program():
    if "nc" not in _cache:
        _cache["nc"] = _build_program()
    return _cache["nc"]


def _coef_blocks(window: int, horizon: int) -> np.ndarray:
    t = np.arange(W, dtype=np.float64)
    t_mean = (window - 1) / 2.0
    tcen = t - t_mean
    denom = (tcen * tcen).sum()
    t_pred = window + horizon - 1
    g = 1.0 / window + tcen * (t_pred - t_mean) / denom  # [W]

    # SBUF layout [k, s*BPC + m]: lhsT for segment s is coef[:, s*BPC:(s+1)*BPC]
    # with coef[m*WSEG + w_in, s*BPC + m] = g[s*WSEG + w_in]
    coef = np.zeros((128, NSEG * BPC), np.float16)
    for s in range(NSEG):
        for m in range(BPC):
            coef[m * WSEG:(m + 1) * WSEG, s * BPC + m] = g[
                s * WSEG:(s + 1) * WSEG
            ].astype(np.float16)
    return coef


def kernel(x: np.ndarray, window, horizon) -> np.ndarray:
    from concourse.bass_utils import run_bass_kernel_spmd

    window = int(window)
    horizon = int(horizon)
    assert x.shape == (B, W, C), x.shape

    nc = _get_program()
    x16 = np.ascontiguousarray(x, dtype=np.float16)
    coef = _coef_blocks(window, horizon)

    in_maps = [
        {"x": x16[c * BPC:(c + 1) * BPC], "coef": coef} for c in range(NCORES)
    ]
    res = run_bass_kernel_spmd(nc, in_maps, list(range(NCORES)))
    out = np.concatenate([res.results[c]["out"] for c in range(NCORES)], axis=0)
    return out.astype(np.float32)
